# revision 32
# baseline (speedup 1.0000x reference)
"""Trainium2 Bass kernel for SAGAN-style self-attention (nn_Attention_13056700580138).

Reference computation (per batch element, with N = H*W = 4096, C = 256, CK = 32):
    f  = x @ Wf + bf            [N, CK]
    g  = x @ Wg + bg            [N, CK]
    hh = x @ Wh + bh            [N, C]
    S  = g @ f^T                [N, N]
    A  = softmax(S, axis=-1)
    o  = A @ hh                 [N, C]
    out = gamma * (o @ Wo + bo) + x

Sharding: data-parallel over batch - one batch element per NeuronCore (B = 8 = n_cores).

Per-core strategy:
  * Attention matmuls (scores, A@hw) run as float32r (FP22 operand reads, fp32
    accumulate): 1 cycle/row on the PE at moving-dim >= 256 - 4x faster than
    true fp32. This is the PE cost floor: fp8 DoubleRow (0.5 cyc/row) was
    measured numerically infeasible (softmax exp range needs a per-query max
    that costs >= a full extra engine pass).
  * Host folds: Whw = gamma*(Wh@Wo) (gamma folded so the epilogue is one
    reciprocal + one fused multiply-add), and the row-bias bhw = bh@Wo + bo
    passes through the row-stochastic softmax exactly, so it is folded into
    the residual on the host: x_res = x + gamma*bhw.
  * x is staged twice: residual-biased pixel-major (epilogue, fp32) and
    TRANSPOSED on the host as bf16 (xT, [C, N]) - no on-chip PE transposes,
    and the 4MB->2MB stream keeps the serial ~360B/ns DMA pipe subcritical
    during the prologue. The projection weights are bf16 too (walrus requires
    fp32/fp32r operands to pair only with themselves). Verified numerics:
    maxrel 0.0124 vs the 2e-2 gate on the fixed seed-0 inputs.
  * Scores are computed transposed (S^T tiles [128 keys, 512 queries]) so the
    exp'd tiles feed the A @ hw accumulation directly as stationary operands.
  * The CK=32 score contraction would idle 3/4 of the PE array, so 4 key blocks
    run concurrently in separate tile_position row groups (full array on HW).
  * Key block b = 4*g4 + t lives in row group t at column g4, so f/g DMA
    re-layouts complete in two batches and attention never waits on a
    monolithic end-of-prologue DMA. DMA count is minimized throughout: the
    timeline model serializes every descriptor through one ~625ns HWDGE slot.
  * Softmax needs no max-subtraction (|scores| < ~60 by construction, exp fits
    fp32) and no N x N normalize pass: an all-ones column appended to the value
    matrix makes the same accumulation emit the softmax row-sums; one reciprocal
    + fused multiply-add per [128, 256] output block finishes softmax+residual.
  * exp runs on [128, 1024] PSUM score pairs (2 banks); score pairs (2 bufs) +
    o-accumulators (4) exactly fill the 8 PSUM banks.
  * The attention inner loop is software-pipelined one round: PE emits round r
    scores, then round r-1's A@hw matmuls, while the Activation engine exps
    round r in the shadow - the PE never waits on exp in steady state.
"""

from contextlib import ExitStack

import ml_dtypes as _ml
import numpy as np

import bass_rust
import concourse.bass as bass
import concourse.mybir as mybir
import concourse.tile as tile
from concourse.bass_utils import run_bass_kernel_spmd
from concourse.vector_clock import ScopedClock

FP = mybir.dt.float32
FPR = mybir.dt.float32r
BF = mybir.dt.bfloat16
AF = mybir.ActivationFunctionType
ALU = mybir.AluOpType

B, H, W, C = 8, 64, 64, 256
CK = C // 8
N = H * W  # 4096
NCORES = 8


# --- workaround: walrus in this toolchain lowers at most one sync-wait per SP
# CTRL instruction, but TileContext's final drain carries one wait per busy
# processor. Split them across single-wait carrier nops (same engine queue,
# program order => identical semantics).
def _split_drain_and_barrier(self, tick_clock, wait_clock):
    nc = self.nc
    ticks = list(eval(repr(tick_clock.global_clock).replace("VectorClock", "")))
    nproc = len(ticks)
    for i, t in enumerate(ticks):
        if t > 0:
            sub = [0] * nproc
            sub[i] = t
            carrier = nc.sync.nop(nofuse=True, hint="drain_split_wait")
            wait_clock.add_sem_waits(
                carrier.ins, ScopedClock({None: bass_rust.VectorClock(sub)})
            )
    nc.sync.drain()
    nc.all_engine_barrier()
    assert self.sems is not None
    popped = nc._tile_sem_poison_stack.pop()
    assert popped is self._sem_poison
    nc.clear_and_free_semaphores(list(self.sems.allocated().values()))
    nc.all_engine_barrier()


tile.TileContext._drain_and_barrier = _split_drain_and_barrier


def _split_instruction_waits(nc):
    """walrus in this toolchain lowers at most one sync-wait per instruction
    for several instruction templates. After Tile scheduling, move any extra
    waits onto single-wait carrier nops inserted just before the instruction
    on the same engine queue (identical blocking semantics)."""
    cnt = 0
    for fn in nc.m.functions:
        for bb in fn.blocks:
            out = []
            changed = False
            for ins in bb.instructions:
                si = ins.sync_info
                waits = list(si.on_wait) if (si is not None and si.on_wait) else []
                if len(waits) > 1:
                    changed = True
                    for wx in waits[:-1]:
                        nop = mybir.InstNoOp(name=f"wsplit-{cnt}", ins=[], outs=[])
                        cnt += 1
                        nop.engine = ins.engine
                        nop.sync_info = mybir.SyncInfo(on_wait=[wx], on_update=[])
                        nc.register_instruction(nop, overwrite=True)
                        out.append(nop)
                    si.on_wait = [waits[-1]]
                out.append(ins)
            if changed:
                bb.instructions = out


def _emit(ctx, nc, tc, t_in, t_out):
    singles = ctx.enter_context(tc.tile_pool(name="singles", bufs=1))
    etp = ctx.enter_context(tc.tile_pool(name="etp", bufs=6))
    work = ctx.enter_context(tc.tile_pool(name="work", bufs=4))

    # ---------------- input staging (DMA count is precious) ---------------
    # xT [C, N] host-transposed; [128, 2(kc), 8(s), 512].  First two DMAs
    # cover slices 0-1 of both kc halves so the first f matmul starts early.
    # weights first: the DMA pipe is serial, nothing computes until these land.
    # wpack = [Wfg | Whw] host-concat -> one dispatch covers every weight.
    wpack = singles.tile([128, 2, 2 * CK + C], BF)
    wp_view = t_in["Wpack"].ap().rearrange("(kc p) n -> p kc n", p=128)
    nc.sync.dma_start(out=wpack[:], in_=wp_view[:])
    wfg_sb = wpack[:, :, 0:2 * CK]
    whw_sb = wpack[:, :, 2 * CK:]
    bfg_col = singles.tile([2 * CK, 1], FP)

    xT_sb = singles.tile([128, 2, N], BF)
    xT_view = t_in["xT"].ap().rearrange("(kc p) n -> p kc n", p=128)
    for sp in range(4):  # slice pairs, kc-interleaved so slice s waits only piece s//2
        for kc in range(2):
            nc.sync.dma_start(
                out=xT_sb[:, kc, 1024 * sp:1024 * (sp + 1)],
                in_=xT_view[:, kc, 1024 * sp:1024 * (sp + 1)],
            )
        if sp == 0:
            nc.sync.dma_start(out=bfg_col[:], in_=t_in["bfg"][:, :])

    x_pix = singles.tile([128, 32, C], FP)

    # ---------------- prologue: f/g projections + hw1 ---------------------
    # fT4[32t+d, g4, k] = f^T[d, key 512*g4 + 128*t + k]
    # gT4[32t+d, s, q]  = g^T[d, 512*s + q]   (replicated across t)
    # => attention iteration g4 consumes exactly projection slice g4.
    fT4 = singles.tile([128, 8, 128], FPR)
    gT4 = singles.tile([128, 8, 512], FPR)
    # hw1[:, kb, :] = [x @ Whw | 1] rows of pixel block kb
    hw1 = singles.tile([128, 32, C + 2], FPR)
    ones_stage = singles.tile([128, 64], FP)
    nc.vector.memset(ones_stage[:], 1.0)
    nc.vector.tensor_copy(out=hw1[:, :, C:C + 2], in_=ones_stage[:])

    pre_ctxB = ExitStack()
    psum_preB = pre_ctxB.enter_context(tc.tile_pool(name="psum_preB", bufs=6, space="PSUM"))
    psum_preA = psum_preB

    fgstage = singles.tile([2 * CK, 8, 512], FPR)

    for s in range(8):
        psum_pre = psum_preA if s < 6 else psum_preB
        late = s >= 6
        psfg = psum_pre.tile([2 * CK, 512], FP, tag="pre", name=f"psfg{s}")
        for kc in range(2):
            nc.tensor.matmul(
                psfg[:], wfg_sb[:, kc, :], xT_sb[:, kc, 512 * s:512 * (s + 1)],
                start=(kc == 0), stop=(kc == 1),
            )
        nc.vector.tensor_scalar_add(
            out=fgstage[:, s, 0:256], in0=psfg[:, 0:256], scalar1=bfg_col[:])
        nc.scalar.activation(
            out=fgstage[:, s, 256:512], in_=psfg[:, 256:512],
            func=AF.Identity, bias=bfg_col[:])
        for tp in range(2):  # hw1 block pairs: one wide copy per pair
            kb0 = 4 * s + 2 * tp
            psh = psum_pre.tile([128, 2, C], FP, tag="pre", name=f"psh{kb0}")
            for t2 in range(2):
                for kc in range(2):
                    nc.tensor.matmul(
                        psh[:, t2, :],
                        xT_sb[:, kc, 128 * (kb0 + t2):128 * (kb0 + t2 + 1)],
                        whw_sb[:, kc, :],
                        start=(kc == 0), stop=(kc == 1),
                    )
            if tp == 0:
                nc.scalar.activation(
                    out=hw1[:, kb0:kb0 + 2, 0:C], in_=psh[:], func=AF.Identity, bias=0.0)
            else:
                nc.vector.tensor_copy(out=hw1[:, kb0:kb0 + 2, 0:C], in_=psh[:])
        if s == 0:
            for t in range(4):
                nc.sync.dma_start(out=gT4[32 * t:32 * (t + 1), 0, :], in_=fgstage[CK:2 * CK, 0, :])
        if s == 3 or s == 7:
            lo = 0 if s == 3 else 4
            for t in range(4):
                nc.sync.dma_start(
                    out=fT4[32 * t:32 * (t + 1), lo:lo + 4, :],
                    in_=fgstage[0:CK, lo:lo + 4, 128 * t:128 * (t + 1)],
                )
            if s == 7:
                for t in range(4):
                    nc.sync.dma_start(out=gT4[32 * t:32 * (t + 1), 1:8, :], in_=fgstage[CK:2 * CK, 1:8, :])

    pre_ctxB.close()
    # x residual loads late: needed first at the qs=0 epilogue, and the serial
    # DMA pipe must first deliver xT + fT4/gT4 batches
    x_view = t_in["xres"].ap().rearrange("(t p) c -> p t c", p=128)
    with tc.high_priority(offset=-500):
        for piece in range(4):
            nc.sync.dma_start(
                out=x_pix[:, 8 * piece:8 * (piece + 1), :],
                in_=x_view[:, 8 * piece:8 * (piece + 1), :],
            )
    psum_sc = ctx.enter_context(tc.tile_pool(name="psum_sc", bufs=2, space="PSUM"))
    psum_o = ctx.enter_context(tc.tile_pool(name="psum_o", bufs=4, space="PSUM"))

    # ---------------- attention (software-pipelined one round) ------------
    # One flat 64-round stream; round r = (qs, g4) emits scores(r) then the
    # A@hw matmuls of round r-1, so the exp of round r always runs in the PE
    # shadow - including across qs boundaries. Each qs's final o-round is
    # j-major and immediately followed by its epilogue.
    def emit_scores(qs, g4):
        et = []
        for half in range(2):
            sc = psum_sc.tile([128, 2, 512], FP, tag="score", name=f"sc_{qs}_{g4}_{half}")
            for tt in range(2):
                t = 2 * half + tt
                nc.tensor.matmul(
                    sc[:, tt, :],
                    fT4[32 * t:32 * (t + 1), g4, :],
                    gT4[32 * t:32 * (t + 1), qs, :],
                    start=True, stop=True,
                    tile_position=(32 * t, 0),
                )
            e = etp.tile([128, 2, 512], FPR, tag="et", name=f"et_{qs}_{g4}_{half}")
            nc.scalar.activation(out=e[:], in_=sc[:], func=AF.Exp)
            et.append(e)
        return et

    def emit_oacc(o_ps, et, g4):
        if g4 == 7:  # j-major: j=0 finishes first so its epilogue starts early
            for j in range(4):
                for t in range(4):
                    nc.tensor.matmul(
                        o_ps[j][:],
                        et[t // 2][:, t % 2, 128 * j:128 * (j + 1)],
                        hw1[:, 4 * g4 + t, :],
                        start=False, stop=(t == 3),
                    )
            return
        for t in range(4):
            kb = 4 * g4 + t
            for j in range(4):
                nc.tensor.matmul(
                    o_ps[j][:],
                    et[t // 2][:, t % 2, 128 * j:128 * (j + 1)],
                    hw1[:, kb, :],
                    start=(g4 == 0 and t == 0),
                    stop=False,
                )

    out_view = t_out.ap().rearrange("(q j p) c -> p q j c", p=128, j=4)

    def emit_epilogue(qs, o_ps):
        last = qs == 7
        ostage = None if last else work.tile([128, 4, C], FP, tag="ostage", name=f"ost_{qs}")
        for j in range(4):
            blk = 4 * qs + j
            rinv = work.tile([128, 1], FP, tag="rinv", name=f"rinv_{blk}")
            nc.vector.reciprocal(out=rinv[:], in_=o_ps[j][:, C:C + 1])
            if last:
                if j % 2 == 0:
                    dstpair = work.tile([128, 2, C], FP, tag="outsb", name=f"osb_{blk}")
                out_ap = dstpair[:, j % 2, :]
            else:
                out_ap = ostage[:, j, :]
            nc.vector.scalar_tensor_tensor(
                out=out_ap,
                in0=o_ps[j][:, 0:C],
                scalar=rinv[:],
                in1=x_pix[:, blk, :],
                op0=ALU.mult,
                op1=ALU.add,
            )
            if last and j % 2 == 1:
                nc.sync.dma_start(
                    out=out_view[:, 7, j - 1:j + 1, :],
                    in_=dstpair[:],
                )
        if not last:
            nc.sync.dma_start(out=out_view[:, qs, :, :], in_=ostage[:])

    o_tiles = {}
    prev = None
    for r in range(64):
        qs, g4 = divmod(r, 8)
        if g4 == 0:
            o_tiles[qs] = [
                psum_o.tile([128, C + 2], FP, tag="oacc", name=f"oacc_{qs}_{j}")
                for j in range(4)
            ]
        cur = emit_scores(qs, g4)
        if r > 0:
            pqs, pg4 = divmod(r - 1, 8)
            emit_oacc(o_tiles[pqs], prev, pg4)
            if pg4 == 7:
                emit_epilogue(pqs, o_tiles.pop(pqs))
        prev = cur
    emit_oacc(o_tiles[7], prev, 7)
    emit_epilogue(7, o_tiles.pop(7))


_CACHE = {}


def _build():
    if "nc" not in _CACHE:
        nc = bass.Bass("TRN2", target_bir_lowering=False, debug=False)
        t_in = {
            "xres": nc.dram_tensor("xres", [N, C], FP, kind="ExternalInput"),
            "xT": nc.dram_tensor("xT", [C, N], BF, kind="ExternalInput"),
            "Wpack": nc.dram_tensor("Wpack", [C, 2 * CK + C], BF, kind="ExternalInput"),
            "bfg": nc.dram_tensor("bfg", [2 * CK, 1], FP, kind="ExternalInput"),
        }
        t_out = nc.dram_tensor("out", [N, C], FP, kind="ExternalOutput")
        with tile.TileContext(nc) as tc:
            with ExitStack() as ctx:
                _emit(ctx, nc, tc, t_in, t_out)
        _split_instruction_waits(nc)
        _CACHE["nc"] = nc
    return _CACHE["nc"]


def kernel(x, Wf, bf, Wg, bg, Wh, bh, Wo, bo, gamma, _trace=False, _tmpdir=None):
    nc = _build()
    x = np.ascontiguousarray(np.asarray(x, dtype=np.float32)).reshape(B, N, C)
    Wh = np.asarray(Wh, np.float64)
    Wo = np.asarray(Wo, np.float64)
    gam = np.float32(np.asarray(gamma).ravel()[0])
    whw = np.ascontiguousarray((gam * (Wh @ Wo)).astype(_ml.bfloat16))
    bhw = (np.asarray(bh, np.float64) @ Wo + np.asarray(bo, np.float64)).astype(np.float32)
    wfg = np.ascontiguousarray(
        np.concatenate([np.asarray(Wf, np.float32), np.asarray(Wg, np.float32)], axis=1).astype(_ml.bfloat16))
    bfg = np.ascontiguousarray(
        np.concatenate([np.asarray(bf, np.float32), np.asarray(bg, np.float32)]).reshape(2 * CK, 1))
    w = {
        "Wpack": np.ascontiguousarray(np.concatenate([np.asarray(wfg), np.asarray(whw)], axis=1)),
        "bfg": bfg,
    }
    in_maps = [
        dict(
            w,
            xres=np.ascontiguousarray(x[i] + gam * bhw),
            xT=np.ascontiguousarray(x[i].T.astype(_ml.bfloat16)),
        )
        for i in range(NCORES)
    ]
    res = run_bass_kernel_spmd(
        nc, in_maps, core_ids=list(range(NCORES)), trace=_trace, tmpdir=_tmpdir
    )
    out = np.stack([res.results[i]["out"] for i in range(NCORES)])
    if _trace:
        kernel._last_result = res
    return out.reshape(B, H, W, C).astype(np.float32)
